# revision 18
# baseline (speedup 1.0000x reference)
"""Trainium2 Bass kernel for autoregressive multi-head self-attention.

Problem: B=2, S=2048, H=2048 (16 heads x 128), RoPE, causal softmax with the
(faithful-to-source) sqrt(head_dim) score MULTIPLIER, out projection.

Sharding: 8 cores = 2 (batch) x 4 (head-groups of 4 heads). Attention is fully
local per core. Out-proj is row-parallel: each core emits a partial [S, H]
output; host sums the 4 partials per batch element.

All matmuls run as float32r (fp32 storage, reduced-precision PE fast path,
1 cycle/row at N>=256). Softmax is exact fp32 on DVE/ACT.
"""

import math
import sys

sys.path.insert(0, "/opt/trn_rl_repo")

import numpy as np

import concourse.bacc as bacc
import concourse.tile as tile
from concourse import bass_utils, mybir
from contextlib import ExitStack

P = 128          # partitions / head dim / q,k,v tile
S = 2048         # sequence length
H = 2048         # hidden
NH = 16          # total heads
HPC = 4          # heads per core
NCORES = 8
SC = 512         # s-chunk width for projections
NCT = H // P     # 16 c-tiles (contraction)
NQT = S // P     # 16 q tiles
NEG = -1.0e30

R32 = mybir.dt.float32r
F32 = mybir.dt.float32
BF16 = mybir.dt.bfloat16
AX = mybir.AxisListType.X
EXP = mybir.ActivationFunctionType.Exp


def _build_program():
    nc = bacc.Bacc("TRN2", target_bir_lowering=False, debug=False)

    xT = nc.dram_tensor("xT", [H, S], R32, kind="ExternalInput")        # x[b].T
    wqT = nc.dram_tensor("wqT", [H, HPC * P], R32, kind="ExternalInput")
    wkT = nc.dram_tensor("wkT", [H, HPC * P], R32, kind="ExternalInput")
    wvT = nc.dram_tensor("wvT", [H, HPC * P], R32, kind="ExternalInput")
    woT = nc.dram_tensor("woT", [HPC * P, H], R32, kind="ExternalInput")
    cosq = nc.dram_tensor("cosq", [P, S], R32, kind="ExternalInput")    # * sqrt(hd)
    sinq = nc.dram_tensor("sinq", [P, S], R32, kind="ExternalInput")    # * sqrt(hd)
    cosk = nc.dram_tensor("cosk", [P, S], R32, kind="ExternalInput")
    sink = nc.dram_tensor("sink", [P, S], R32, kind="ExternalInput")
    permT = nc.dram_tensor("permT", [P, P], R32, kind="ExternalInput")  # rot-half
    maskc = nc.dram_tensor("maskc", [P, P], F32, kind="ExternalInput")  # causal add
    out = nc.dram_tensor("out", [S, H], F32, kind="ExternalOutput")     # partial

    with tile.TileContext(nc) as tc, ExitStack() as ctx:
        cpool = ctx.enter_context(tc.tile_pool(name="consts", bufs=1))
        mask_sb = cpool.tile([P, P], F32, tag="mask", name="mask_sb")
        perm_sb = cpool.tile([P, P], R32, tag="perm", name="perm_sb")
        nc.gpsimd.dma_start(out=mask_sb, in_=maskc.ap())
        nc.gpsimd.dma_start(out=perm_sb, in_=permT.ap())

        # ctxT[h]: [d=128, S] per head, alive until the out-projection
        ctxpool = ctx.enter_context(tc.tile_pool(name="ctxp", bufs=1))
        ctxT = [
            ctxpool.tile([P, S], R32, tag=f"ctxT{h}", name=f"ctxT{h}")
            for h in range(HPC)
        ]

        for hp in range(2):  # head-pair passes: heads {2hp, 2hp+1}
            with ExitStack() as pctx:
                wpool = pctx.enter_context(tc.tile_pool(name=f"w{hp}", bufs=1))
                wq_sb = wpool.tile([P, NCT, 2 * P], R32, tag="wq", name=f"wq{hp}")
                wk_sb = wpool.tile([P, NCT, 2 * P], R32, tag="wk", name=f"wk{hp}")
                wv_sb = wpool.tile([P, NCT, 2 * P], R32, tag="wv", name=f"wv{hp}")
                osl = slice(hp * 2 * P, (hp + 1) * 2 * P)
                for wsb, wdr in ((wq_sb, wqT), (wk_sb, wkT), (wv_sb, wvT)):
                    nc.sync.dma_start(
                        out=wsb,
                        in_=wdr.ap()[:, osl].rearrange("(t p) o -> p t o", p=P),
                    )

                qkvpool = pctx.enter_context(tc.tile_pool(name=f"qkv{hp}", bufs=1))
                qT = [
                    qkvpool.tile([P, S], R32, tag=f"qT{i}", name=f"qT{hp}_{i}")
                    for i in range(2)
                ]
                kT = [
                    qkvpool.tile([P, S], R32, tag=f"kT{i}", name=f"kT{hp}_{i}")
                    for i in range(2)
                ]
                v_sb = qkvpool.tile([P, NQT, 2 * P], BF16, tag="v", name=f"v{hp}")

                # ---- Phase A: projections + RoPE ----
                with ExitStack() as actx:
                    xpool = actx.enter_context(tc.tile_pool(name=f"x{hp}", bufs=5))
                    tpool = actx.enter_context(tc.tile_pool(name=f"t{hp}", bufs=5))
                    mpool = actx.enter_context(tc.tile_pool(name=f"m{hp}", bufs=3))
                    pqk = actx.enter_context(
                        tc.tile_pool(name=f"pqk{hp}", bufs=4, space="PSUM")
                    )
                    prot = actx.enter_context(
                        tc.tile_pool(name=f"prot{hp}", bufs=2, space="PSUM")
                    )
                    ppv = actx.enter_context(
                        tc.tile_pool(name=f"ppv{hp}", bufs=2, space="PSUM")
                    )

                    def rope(dest, ps, cos_t, sin_t, nm):
                        raw = mpool.tile([P, SC], R32, tag="qraw", name=f"raw{nm}")
                        nc.scalar.copy(out=raw, in_=ps)
                        rot = prot.tile([P, SC], F32, tag="rot", name=f"rot{nm}")
                        nc.tensor.matmul(
                            rot, lhsT=(perm_sb), rhs=(raw), start=True, stop=True
                        )
                        nc.vector.tensor_mul(out=dest, in0=raw, in1=cos_t)
                        tmp = mpool.tile([P, SC], R32, tag="rtmp", name=f"tmp{nm}")
                        nc.vector.tensor_mul(out=tmp, in0=rot, in1=sin_t)
                        nc.vector.tensor_add(out=dest, in0=dest, in1=tmp)

                    for sc in range(S // SC):
                        ssl = slice(sc * SC, (sc + 1) * SC)
                        xg = []
                        for g in range(4):  # 4 c-tiles per DMA
                            t = xpool.tile(
                                [P, 4, SC], R32, tag="xb", name=f"xb{hp}_{sc}_{g}"
                            )
                            nc.sync.dma_start(
                                out=t,
                                in_=xT.ap()[g * 4 * P : (g + 1) * 4 * P, ssl]
                                .rearrange("(t p) s -> p t s", p=P),
                            )
                            xg.append(t)
                        xb = [xg[ct // 4][:, ct % 4, :] for ct in range(NCT)]
                        trig = {}
                        for tn, tdr in (
                            ("cq", cosq), ("sq", sinq), ("ck", cosk), ("sk", sink)
                        ):
                            t = tpool.tile(
                                [P, SC], R32, tag="trig", name=f"{tn}{hp}_{sc}"
                            )
                            nc.gpsimd.dma_start(out=t, in_=tdr.ap()[:, ssl])
                            trig[tn] = t

                        for hh in range(2):
                            hsl = slice(hh * P, (hh + 1) * P)
                            psq = pqk.tile([P, SC], F32, tag="pqk", name=f"psq{sc}{hh}")
                            psk = pqk.tile([P, SC], F32, tag="pqk", name=f"psk{sc}{hh}")
                            for ct in range(NCT):
                                nc.tensor.matmul(
                                    psq,
                                    lhsT=(wq_sb[:, ct, hsl]),
                                    rhs=(xb[ct]),
                                    start=(ct == 0),
                                    stop=(ct == NCT - 1),
                                )
                                nc.tensor.matmul(
                                    psk,
                                    lhsT=(wk_sb[:, ct, hsl]),
                                    rhs=(xb[ct]),
                                    start=(ct == 0),
                                    stop=(ct == NCT - 1),
                                )
                            rope(qT[hh][:, ssl], psq, trig["cq"], trig["sq"],
                                 f"q{hp}{sc}{hh}")
                            rope(kT[hh][:, ssl], psk, trig["ck"], trig["sk"],
                                 f"k{hp}{sc}{hh}")

                        for sti in range(SC // P):
                            st = sc * (SC // P) + sti
                            psv = ppv.tile([P, 2 * P], F32, tag="pv", name=f"pv{st}")
                            for ct in range(NCT):
                                nc.tensor.matmul(
                                    psv,
                                    lhsT=(xb[ct][:, sti * P : (sti + 1) * P]),
                                    rhs=(wv_sb[:, ct, :]),
                                    start=(ct == 0),
                                    stop=(ct == NCT - 1),
                                )
                            nc.any.tensor_copy(out=v_sb[:, st, :], in_=psv)

                # ---- Phase C: attention ----
                with ExitStack() as cctx:
                    ppool = cctx.enter_context(tc.tile_pool(name=f"pr{hp}", bufs=4))
                    ptapool = cctx.enter_context(tc.tile_pool(name=f"pt{hp}", bufs=6))
                    smpool = cctx.enter_context(tc.tile_pool(name=f"sm{hp}", bufs=4))
                    psc_pool = cctx.enter_context(
                        tc.tile_pool(name=f"psc{hp}", bufs=3, space="PSUM")
                    )
                    pcx_pool = cctx.enter_context(
                        tc.tile_pool(name=f"pcx{hp}", bufs=2, space="PSUM")
                    )

                    for hh in range(2):
                        h = hp * 2 + hh
                        for Q in range(4):  # q-supertiles of 512
                            probs = []
                            ptas = []
                            for qi_in in range(4):
                                qi = Q * 4 + qi_in
                                L = (qi + 1) * P
                                CH = 1024
                                nch = (L + CH - 1) // CH
                                pt = ppool.tile(
                                    [P, S], BF16, tag="probs",
                                    name=f"pr{hp}{hh}{Q}{qi_in}",
                                )
                                probs.append(pt)
                                maxp = smpool.tile(
                                    [P, 2], F32, tag="maxp", name=f"mx{h}{qi}"
                                )
                                pscs = []
                                for cn in range(nch):
                                    n0 = cn * CH
                                    w = min(L, n0 + CH) - n0
                                    psc = psc_pool.tile(
                                        [P, CH], F32, tag="sc", name=f"sc{h}{qi}{cn}"
                                    )
                                    pscs.append((psc, n0, w))
                                    # QK in 512-wide matmuls (one PSUM bank each)
                                    for j0 in range(0, w, 512):
                                        jw = min(w, j0 + 512) - j0
                                        nc.tensor.matmul(
                                            psc[:, j0 : j0 + jw],
                                            lhsT=(qT[hh][:, qi * P : (qi + 1) * P]),
                                            rhs=(kT[hh][:, n0 + j0 : n0 + j0 + jw]),
                                            start=True,
                                            stop=True,
                                        )
                                # causal mask on the diagonal block (last 128 cols)
                                psc_d, n0_d, w_d = pscs[-1]
                                off = qi * P - n0_d
                                nc.vector.tensor_add(
                                    out=psc_d[:, off : off + P],
                                    in0=psc_d[:, off : off + P],
                                    in1=mask_sb,
                                )
                                for cn, (psc, n0, w) in enumerate(pscs):
                                    nc.vector.reduce_max(
                                        out=maxp[:, cn : cn + 1],
                                        in_=psc[:, :w],
                                        axis=AX,
                                        negate=(nch == 1),
                                    )
                                if nch == 1:
                                    rowneg = maxp[:, 0:1]
                                else:
                                    rowneg = smpool.tile(
                                        [P, 1], F32, tag="rneg", name=f"rn{h}{qi}"
                                    )
                                    nc.vector.reduce_max(
                                        out=rowneg, in_=maxp[:, :nch], axis=AX,
                                        negate=True,
                                    )
                                sums = smpool.tile(
                                    [P, 2], F32, tag="sums", name=f"sm{h}{qi}"
                                )
                                for cn, (psc, n0, w) in enumerate(pscs):
                                    nc.scalar.activation(
                                        out=pt[:, n0 : n0 + w],
                                        in_=psc[:, :w],
                                        func=EXP,
                                        bias=rowneg,
                                        scale=1.0,
                                        accum_out=sums[:, cn : cn + 1],
                                    )
                                ssum = smpool.tile(
                                    [P, 1], F32, tag="ssum", name=f"ss{h}{qi}"
                                )
                                nc.vector.reduce_sum(
                                    out=ssum, in_=sums[:, :nch], axis=AX
                                )
                                recip = smpool.tile(
                                    [P, 1], F32, tag="recip", name=f"rc{h}{qi}"
                                )
                                nc.vector.reciprocal(out=recip, in_=ssum)
                                nc.vector.tensor_scalar_mul(pt[:, :L], pt[:, :L], recip)

                                # one batched xbar transpose: [q, L] -> [k, kt, q]
                                pta = ptapool.tile(
                                    [P, NQT, P], BF16, tag="pta",
                                    name=f"pta{h}{qi}",
                                )
                                ptas.append(pta)
                                eng = nc.scalar  # ACT-HWDGE: xbar transposes only
                                eng.dma_start(
                                    out=pta[:, : qi + 1, :],
                                    in_=pt[:, :L],
                                    transpose=True,
                                )

                            # PV over k-tiles; per-q-column accumulation groups
                            ctps = pcx_pool.tile(
                                [P, 512], F32, tag="cx", name=f"cx{h}{Q}"
                            )
                            nkt = Q * 4 + 4
                            for kt in range(nkt):
                                vblk = v_sb[:, kt, hh * P : (hh + 1) * P]
                                for qi_in in range(4):
                                    qi = Q * 4 + qi_in
                                    if qi >= kt:
                                        nc.tensor.matmul(
                                            ctps[:, qi_in * P : (qi_in + 1) * P],
                                            lhsT=vblk,
                                            rhs=ptas[qi_in][:, kt, :],
                                            start=(kt == 0 and qi_in == 0),
                                            stop=(kt == qi),
                                            skip_group_check=True,
                                        )
                            nc.any.tensor_copy(
                                out=ctxT[h][:, Q * 512 : (Q + 1) * 512], in_=ctps
                            )

        # ---- Phase D: out projection (partial sums over this core's channels)
        with ExitStack() as dctx:
            wopool = dctx.enter_context(tc.tile_pool(name="wo", bufs=1))
            ostpool = dctx.enter_context(tc.tile_pool(name="ost", bufs=3))
            po_pool = dctx.enter_context(
                tc.tile_pool(name="po", bufs=8, space="PSUM")
            )
            wo_sb = wopool.tile([P, HPC, H], R32, tag="wo", name="wo_sb")
            nc.sync.dma_start(
                out=wo_sb, in_=woT.ap().rearrange("(t p) o -> p t o", p=P)
            )
            for st in range(NQT):
                psos = [
                    po_pool.tile([P, 512], F32, tag="po", name=f"po{st}{oc}")
                    for oc in range(4)
                ]
                for h in range(HPC):
                    for oc in range(4):
                        nc.tensor.matmul(
                            psos[oc],
                            lhsT=(ctxT[h][:, st * P : (st + 1) * P]),
                            rhs=(wo_sb[:, h, oc * 512 : (oc + 1) * 512]),
                            start=(h == 0),
                            stop=(h == HPC - 1),
                        )
                ost = ostpool.tile([P, H], F32, tag="ost", name=f"ost{st}")
                for oc in range(4):
                    nc.any.tensor_copy(
                        out=ost[:, oc * 512 : (oc + 1) * 512], in_=psos[oc]
                    )
                nc.sync.dma_start(
                    out=out.ap()[st * P : (st + 1) * P, :], in_=ost
                )

    nc.compile()
    return nc


_NC_CACHE = None


def _get_program():
    global _NC_CACHE
    if _NC_CACHE is None:
        _NC_CACHE = _build_program()
    return _NC_CACHE


def _host_inputs(x, Wq, Wk, Wv, Wo, cos, sin):
    """Build the 8 per-core input maps (host-side sharding + layout prep)."""
    B = x.shape[0]
    sq = math.sqrt(P)

    cosT = np.ascontiguousarray(cos[:S].T.astype(np.float32))  # [128, S]
    sinT = np.ascontiguousarray(sin[:S].T.astype(np.float32))

    # rotate-half as a signed permutation: rot[d] = sign(d) * x[(d+64) % 128]
    perm = np.zeros((P, P), np.float32)
    for d in range(P):
        perm[d, (d + P // 2) % P] = -1.0 if d < P // 2 else 1.0
    permT_np = np.ascontiguousarray(perm.T)

    mask_np = np.triu(np.full((P, P), NEG, np.float32), k=1)

    xTb = [np.ascontiguousarray(x[b].T.astype(np.float32)) for b in range(B)]

    in_maps = []
    for core in range(NCORES):
        b = core // 4
        hg = core % 4
        rows = slice(hg * HPC * P, (hg + 1) * HPC * P)
        in_maps.append(
            {
                "xT": xTb[b],
                "wqT": np.ascontiguousarray(Wq[rows, :].T.astype(np.float32)),
                "wkT": np.ascontiguousarray(Wk[rows, :].T.astype(np.float32)),
                "wvT": np.ascontiguousarray(Wv[rows, :].T.astype(np.float32)),
                "woT": np.ascontiguousarray(Wo[:, rows].T.astype(np.float32)),
                "cosq": np.ascontiguousarray(cosT * sq),
                "sinq": np.ascontiguousarray(sinT * sq),
                "cosk": cosT,
                "sink": sinT,
                "permT": permT_np,
                "maskc": mask_np,
            }
        )
    return in_maps


def kernel(x, Wq, Wk, Wv, Wo, cos, sin, _trace=False):
    nc = _get_program()
    in_maps = _host_inputs(x, Wq, Wk, Wv, Wo, cos, sin)
    res = bass_utils.run_bass_kernel_spmd(
        nc, in_maps, core_ids=list(range(NCORES)), trace=_trace
    )
    kernel.last_result = res
    B = x.shape[0]
    out = np.zeros((B, S, H), np.float32)
    for core in range(NCORES):
        out[core // 4] += res.results[core]["out"]
    return out
